# revision 5
# baseline (speedup 1.0000x reference)
"""ColBERT late-interaction scoring kernel for Trainium2 (Bass/Tile), v12.

v5 + io-wake reduction:
  - Q and D share ONE bf16 tile and ONE xbar transpose (f = (x, b, t, d)
    -> out[d, x*64 + b*4 + t, p]: Q groups 0-63, D groups 64-127).
  - loads half-staged through one 32KB f32 tile (saves SBUF).
  - fp8 packs (QTc8/DSQc8) double-buffered: rep r+1's io fully overlaps
    rep r's matmul rounds; only first-matmul-waits-pack remains serial.
  - QSQ in fp8 (SBUF budget); qsq error ~1e-3, budget 2e-2.
"""

import numpy as np

B, LQ, LD, D = 128, 512, 512, 128
N_CORES = 8
BPC = B // N_CORES
NT = LQ // 128

_compiled = {}


def _split_multi_waits(nc):
    """walrus accepts only ONE sem-wait per instruction; split extras onto
    same-engine NoOps placed just before the instruction."""
    import concourse.mybir as mybir

    for f in nc.m.functions:
        for blk in f.blocks:
            il = blk.instructions
            i = 0
            while i < len(il):
                inst = il[i]
                si = inst.sync_info
                waits = list(si.on_wait) if si and si.on_wait else []
                if len(waits) > 1:
                    for w in waits[:-1]:
                        nop = mybir.InstNoOp(
                            name=nc.get_next_instruction_name(), ins=[], outs=[]
                        )
                        nop.engine = inst.engine
                        nop.sync_info = mybir.SyncInfo(on_wait=[w], on_update=[])
                        il.insert(i, nop)
                        i += 1
                    inst.sync_info = mybir.SyncInfo(
                        on_wait=[waits[-1]], on_update=si.on_update
                    )
                i += 1


def _build(reps: int = 1):
    import concourse.bass as bass
    import concourse.mybir as mybir
    import concourse.tile as tile

    nc = bass.Bass()
    f32 = mybir.dt.float32
    bf16 = mybir.dt.bfloat16
    fp8 = mybir.dt.float8e4

    qe = nc.dram_tensor("qe", [BPC, LQ, D], f32, kind="ExternalInput")
    de = nc.dram_tensor("de", [BPC, LD, D], f32, kind="ExternalInput")
    out = nc.dram_tensor("out", [1, BPC], f32, kind="ExternalOutput")

    with tile.TileContext(nc) as tc:
        with (
            tc.tile_pool(name="consts", bufs=1) as cpool,
            tc.tile_pool(name="work", bufs=1) as wpool,
            tc.tile_pool(name="ps", bufs=2, space="PSUM") as pspool,
        ):
            ones_col = cpool.tile([128, 1], f32)
            nc.gpsimd.memset(ones_col, 1.0)
            MX = cpool.tile([128, BPC * NT], f32)

            for rep in range(reps):
                # 1+2. half-staged loads + casts into ONE combined bf16 tile
                # bQD[p, x, b, t, d] (x=0: Q, x=1: D)
                bQD = wpool.tile([128, 2, BPC, NT, 128], bf16, tag="bQD")
                stg = wpool.tile([128, BPC, NT, 128], f32, tag="stg")
                nc.sync.dma_start(stg, qe.rearrange("b (p t) d -> p b t d", t=NT))
                nc.gpsimd.tensor_copy(bQD[:, 0], stg)
                stg2 = wpool.tile([128, BPC, NT, 128], f32, tag="stg")
                nc.sync.dma_start(stg2, de.rearrange("b (p t) d -> p b t d", t=NT))
                nc.gpsimd.tensor_copy(bQD[:, 1], stg2)

                # 3. ONE xbar transpose for both tensors:
                # out[f % 128, f // 128, p] = in[p, f], f = (x, b, t, d)
                # -> TQD[d, x*64 + b*4 + t, p]
                TQD = wpool.tile([128, 2 * BPC * NT, 128], bf16, tag="TQD")
                nc.sync.dma_start_transpose(
                    TQD, bQD.rearrange("p x b t d -> p (x b t d)")
                )
                QT_all = TQD.rearrange("d g p -> d (g p)")[:, : BPC * LQ]
                DT_all = TQD.rearrange("d g p -> d (g p)")[:, BPC * LQ :]

                # 4. fp8 packs (double-buffered) + squares
                QTc8 = wpool.tile([128, 2, BPC * LQ], fp8, tag="QTc8", bufs=2)
                DSQc8 = wpool.tile([128, 2, BPC * LD], fp8, tag="DSQc8", bufs=2)
                nc.gpsimd.memset(QTc8[:, 1, :], -0.5)
                nc.gpsimd.tensor_copy(QTc8[:, 0, :], QT_all)
                nc.gpsimd.tensor_copy(DSQc8[:, 0, :], DT_all)
                nc.scalar.activation(
                    DSQc8[:, 1, :], DT_all, mybir.ActivationFunctionType.Square
                )
                QSQ_all = wpool.tile([128, BPC * LQ], fp8, tag="QSQ")
                nc.scalar.activation(
                    QSQ_all, QT_all, mybir.ActivationFunctionType.Square
                )
                qsqd = cpool.tile([128, BPC], f32)
                nc.vector.reduce_sum(
                    qsqd,
                    QSQ_all.rearrange("d (b k) -> d b k", b=BPC),
                    axis=mybir.AxisListType.X,
                )

                # 5. main loop: one fp8 DoubleRow matmul per (b, t); 2 batches
                # per psum generation, ONE DVE reduce per generation.
                for bb in range(BPC // 2):
                    pst = pspool.tile([128, 2, NT, LD], f32, tag="pst", bufs=1)
                    for i in range(2):
                        b = bb * 2 + i
                        rhs = DSQc8[:, :, b * LD : (b + 1) * LD]
                        for t in range(NT):
                            nc.tensor.matmul(
                                pst[:, i, t, :],
                                lhsT=QTc8[:, :, b * LQ + t * 128 : b * LQ + (t + 1) * 128],
                                rhs=rhs,
                                start=True, stop=True,
                                perf_mode=mybir.MatmulPerfMode.DoubleRow,
                            )
                    nc.vector.reduce_max(
                        MX[:, bb * 2 * NT : (bb + 1) * 2 * NT], pst,
                        axis=mybir.AxisListType.X,
                    )

                # 6. endgame
                msum = cpool.tile([128, BPC], f32)
                nc.vector.reduce_sum(
                    msum,
                    MX.rearrange("p (b t) -> p b t", t=NT),
                    axis=mybir.AxisListType.X,
                )
                sc = cpool.tile([128, BPC], f32)
                nc.vector.scalar_tensor_tensor(
                    sc, msum, 2.0, qsqd,
                    op0=mybir.AluOpType.mult, op1=mybir.AluOpType.subtract,
                )
                ps_s = pspool.tile([1, BPC], f32, tag="pst", bufs=1)
                nc.tensor.matmul(ps_s, lhsT=ones_col, rhs=sc, start=True, stop=True)
                score = cpool.tile([1, BPC], f32)
                nc.vector.tensor_copy(score, ps_s)
                nc.sync.dma_start(out[:, :], score)

    _split_multi_waits(nc)
    return nc


def kernel(query_embedding: np.ndarray, document_embedding: np.ndarray) -> np.ndarray:
    from concourse.bass_utils import run_bass_kernel_spmd

    if "nc" not in _compiled:
        _compiled["nc"] = _build()
    nc = _compiled["nc"]

    qe = np.ascontiguousarray(query_embedding, dtype=np.float32)
    de = np.ascontiguousarray(document_embedding, dtype=np.float32)
    in_maps = [
        {"qe": qe[c * BPC : (c + 1) * BPC], "de": de[c * BPC : (c + 1) * BPC]}
        for c in range(N_CORES)
    ]
    res = run_bass_kernel_spmd(nc, in_maps, core_ids=list(range(N_CORES)))
    return np.concatenate(
        [res.results[c]["out"].reshape(BPC) for c in range(N_CORES)]
    ).astype(np.float32)
